# revision 1
# baseline (speedup 1.0000x reference)
"""Trainium2 Bass kernel for nn_ConvSelfAttention (conv_in -> agent-aware attention -> conv_out).

Sharding: head-parallel for conv_in+attention (core i computes the 320 conv_in
output channels belonging to head i, then head i's 64 (h*w) attention problems);
seq-parallel for conv_out (48 frames per core). No cross-device communication;
the one reshard between the two stages happens on the host.

All matmuls run in float32r (full PE rate, ~1e-4 relative error).
"""

import sys

sys.path.insert(0, "/opt/trn_rl_repo")

import numpy as np

import concourse.bacc as bacc
import concourse.tile as tile
import concourse.mybir as mybir
from concourse.bass_utils import run_bass_kernel_spmd

dt = mybir.dt

N_CORES = 8
SEQ = 384
C = 512
H = W = 8
HWP = 64          # h*w spatial positions
NH = 8            # heads
HD = 64           # head dim
EMB = 5           # k_same, k_other, q_same, q_other, v
CO1 = 320         # conv_in output channels per core (= EMB * HD)
QB = 24           # seq chunk for conv_in matmuls
SCALE = 1.0 / 8.0  # 1/sqrt(HD)

F32 = dt.float32
F32R = dt.float32r
BF16 = dt.bfloat16
MMDT = F32R  # matmul operand dtype
NP_MMDT = np.float32


def build_d1(seq=SEQ, qb=QB, repeat=1, parts="all"):
    """Dispatch 1: conv_in (320 channels) + attention for one head.

    Inputs (per core):
      xpad  [4, 128, seq, 100] f32r  - padded input, ci-tile major, 10x10 frames
      w1    [128, 4, 9, 320]   f32r  - conv_in weights, [ci, ci_tile, tap, co]
      b1    [128, 3]           f32   - conv_in bias per co-tile (tile2 padded)
      am    [3, 128, seq]      f32   - attn_mask for this head, q-tile major
      m     [3, 128, seq]      f32   - agent mask (1.0/0.0)
      m1    [3, 128, seq]      f32   - 1 - agent mask
      ident [128, 128]         f32r  - identity for PE transpose
    Output:
      att   [64, 64, seq]      f32   - attention output [p, d, q]
    """
    nc = bacc.Bacc("TRN2", target_bir_lowering=False, debug=False,
                   num_devices=N_CORES)
    xpad = nc.dram_tensor("xpad", [4, 128, seq, 100], MMDT, kind="ExternalInput").ap()
    w1 = nc.dram_tensor("w1", [128, 4, 9, 320], MMDT, kind="ExternalInput").ap()
    b1 = nc.dram_tensor("b1", [128, 3], F32, kind="ExternalInput").ap()
    am = nc.dram_tensor("am", [seq // 128, 128, seq], MMDT, kind="ExternalInput").ap()
    m = nc.dram_tensor("m", [seq // 128, 128, seq], BF16, kind="ExternalInput").ap()
    m1 = nc.dram_tensor("m1", [seq // 128, 128, seq], BF16, kind="ExternalInput").ap()
    ident = nc.dram_tensor("ident", [128, 128], MMDT, kind="ExternalInput").ap()
    att = nc.dram_tensor("att", [HWP, HD, seq], F32, kind="ExternalOutput").ap()

    n_qb = seq // qb
    n_qt = seq // 128  # q tiles for attention

    from contextlib import ExitStack

    def conv_block(nc, P, blk):
        q0 = blk * qb
        slab = P["xslab"].tile([128, 4, qb, 100], MMDT, tag="slab")
        for cit in range(4):
            nc.sync.dma_start(slab[:, cit], xpad[cit, :, q0:q0 + qb, :])
        slab5 = slab[:].rearrange("c t q (y x) -> c t q y x", y=10)
        for cot in range(3):
            co0 = cot * 128
            cw = 128 if cot < 2 else 64
            osb = P["feo"].tile([128, 8, 8, qb], MMDT, tag="osb")
            for yp in range(4):
                ps = P["cps"].tile([128, qb, 2, 8], F32, tag="cps")
                k = 0
                for cit in range(4):
                    for tap in range(9):
                        ddy, ddx = tap // 3 - 1, tap % 3 - 1
                        if parts == "convflat":
                            # timing-only: contiguous 2-dim moving operand
                            rhs = slab[:, cit].rearrange(
                                "c q s -> c (q s)")[:, 0:384]
                        else:
                            rhs = slab5[:, cit, :,
                                        2 * yp + 1 + ddy:2 * yp + 3 + ddy,
                                        1 + ddx:9 + ddx]
                        nc.tensor.matmul(
                            ps[:cw], P["w_sb"][:, cit, tap, co0:co0 + cw],
                            rhs, start=(k == 0), stop=(k == 35))
                        k += 1
                nc.scalar.activation(
                    osb[:cw, 2 * yp:2 * yp + 2, :, :]
                    .rearrange("c y x q -> c q y x"), ps[:cw],
                    mybir.ActivationFunctionType.Identity,
                    bias=P["b_sb"][:cw, cot:cot + 1])
            if P.get("nospill"):
                dst = P["feats2"][blk, cot, :cw].rearrange("c p q -> c (p q)")
                src = osb[:cw].rearrange("c y x q -> c (y x q)")
            else:
                dst = P["feats"][:, co0:co0 + cw, q0:q0 + qb] \
                    .rearrange("p c q -> c p q")
                src = osb[:cw].rearrange("c y x q -> c (y x) q")
            nc.sync.dma_start(dst, src)

    def attn_problem(nc, P, p):
        feats, id_sb = P["feats"], P["id_sb"]
        ft = P["fin"].tile([128, 3, seq], MMDT, tag="ft")
        nc.sync.dma_start(ft[:, 0], feats[p, 0:128, :])
        nc.sync.dma_start(ft[:, 1], feats[p, 128:256, :])
        nc.sync.dma_start(ft[0:64, 2], feats[p, 256:320, :])
        ks, ko = ft[0:64, 0], ft[64:128, 0]
        qs, qo = ft[0:64, 1], ft[64:128, 1]
        v = ft[0:64, 2]

        esb = P["esb"].tile([128, 2, n_qt, seq], F32, tag="esb")
        zsb = P["zsb"].tile([128, 2, n_qt], F32, tag="zsb")
        rz = P["zsb"].tile([128, 2, n_qt], F32, tag="rz")
        for so, (qq, kk) in enumerate([(qs, ks), (qo, ko)]):
            for qt in range(n_qt):
                ps = P["sps"].tile([128, 512], F32, tag="sps")
                # psum = attn_mask (via identity matmul) + Q.K (scale is
                # pre-folded into the q-channel conv weights on the host)
                nc.tensor.matmul(
                    ps[:, :seq], id_sb, P["am_sb"][:, qt],
                    start=True, stop=False)
                nc.tensor.matmul(
                    ps[:, :seq], qq[:, qt * 128:(qt + 1) * 128], kk,
                    start=False, stop=True)
                nc.scalar.activation(
                    esb[:, so, qt], ps[:, :seq],
                    mybir.ActivationFunctionType.Exp,
                    accum_out=zsb[:, so, qt:qt + 1])
        nc.vector.reciprocal(rz[:], zsb[:])

        attn = P["atn"].tile([128, n_qt, seq], MMDT, tag="attn")
        for qt in range(n_qt):
            x1 = P["mix"].tile([128, seq], F32, tag="x1")
            nc.vector.scalar_tensor_tensor(
                x1[:], esb[:, 0, qt], rz[:, 0, qt:qt + 1], P["m_sb"][:, qt],
                op0=mybir.AluOpType.mult, op1=mybir.AluOpType.mult)
            x2 = P["mix"].tile([128, seq], F32, tag="x2")
            nc.vector.scalar_tensor_tensor(
                x2[:], esb[:, 1, qt], rz[:, 1, qt:qt + 1], P["m1_sb"][:, qt],
                op0=mybir.AluOpType.mult, op1=mybir.AluOpType.mult)
            nc.gpsimd.tensor_add(attn[:, qt], x1[:], x2[:])

        # transpose V: [d, k] -> [k, d]
        vsb = P["atn"].tile([128, n_qt, HD], MMDT, tag="vsb")
        for kt in range(n_qt):
            vps = P["vps"].tile([128, HD], MMDT, tag="vps")
            nc.tensor.transpose(
                vps[:], v[:, kt * 128:(kt + 1) * 128], id_sb[0:64, 0:64])
            nc.vector.tensor_copy(vsb[:, kt], vps[:])
        # transpose attn: [q, k] -> [k, q]
        atT = P["atn"].tile([128, n_qt, seq], MMDT, tag="atT")
        for kt in range(n_qt):
            tps = P["tps"].tile([128, 512], MMDT, tag="tps")
            for qt in range(n_qt):
                nc.tensor.transpose(
                    tps[:, qt * 128:(qt + 1) * 128],
                    attn[:, qt, kt * 128:(kt + 1) * 128], id_sb)
            nc.vector.tensor_copy(atT[:, kt], tps[:, :seq])
        # out^T[d, q] = sum_k V^T[d,k] attn^T[k,q]
        avps = P["avps"].tile([HD, 512], F32, tag="avps")
        for kt in range(n_qt):
            nc.tensor.matmul(
                avps[:, :seq], vsb[:, kt], atT[:, kt],
                start=(kt == 0), stop=(kt == n_qt - 1))
        avo = P["avo"].tile([HD, seq], F32, tag="avo")
        nc.scalar.copy(avo[:], avps[:, :seq])
        nc.sync.dma_start(att[p], avo[:])

    with tile.TileContext(nc) as tc, ExitStack() as ctx:
        P = {}
        P["consts"] = ctx.enter_context(tc.tile_pool(name="consts", bufs=1))
        P["dram"] = ctx.enter_context(tc.tile_pool(name="dram", bufs=1, space="DRAM"))
        P["xslab"] = ctx.enter_context(tc.tile_pool(name="xslab", bufs=2))
        P["feo"] = ctx.enter_context(tc.tile_pool(name="feo", bufs=1))
        P["fin"] = ctx.enter_context(tc.tile_pool(name="fin", bufs=2))
        P["esb"] = ctx.enter_context(tc.tile_pool(name="esb", bufs=1))
        P["zsb"] = ctx.enter_context(tc.tile_pool(name="zsb", bufs=2))
        P["mix"] = ctx.enter_context(tc.tile_pool(name="mix", bufs=2))
        P["atn"] = ctx.enter_context(tc.tile_pool(name="atn", bufs=2))
        P["avo"] = ctx.enter_context(tc.tile_pool(name="avo", bufs=2))
        P["cps"] = ctx.enter_context(tc.tile_pool(name="cps", bufs=2, space="PSUM"))
        P["sps"] = ctx.enter_context(tc.tile_pool(name="sps", bufs=2, space="PSUM"))
        P["tps"] = ctx.enter_context(tc.tile_pool(name="tps", bufs=2, space="PSUM"))
        P["avps"] = ctx.enter_context(tc.tile_pool(name="avps", bufs=1, space="PSUM"))
        P["vps"] = ctx.enter_context(tc.tile_pool(name="vps", bufs=1, space="PSUM"))

        # ---- load constants ----
        P["w_sb"] = P["consts"].tile([128, 4, 9, CO1], MMDT, tag="w_sb", name="w_sb")
        nc.sync.dma_start(P["w_sb"][:], w1)
        P["b_sb"] = P["consts"].tile([128, 3], F32, tag="b_sb", name="b_sb")
        nc.sync.dma_start(P["b_sb"][:], b1)
        P["am_sb"] = P["consts"].tile([128, n_qt, seq], MMDT, tag="am_sb", name="am_sb")
        P["m_sb"] = P["consts"].tile([128, n_qt, seq], BF16, tag="m_sb", name="m_sb")
        P["m1_sb"] = P["consts"].tile([128, n_qt, seq], BF16, tag="m1_sb", name="m1_sb")
        for qt in range(n_qt):
            nc.sync.dma_start(P["am_sb"][:, qt], am[qt])
            nc.sync.dma_start(P["m_sb"][:, qt], m[qt])
            nc.sync.dma_start(P["m1_sb"][:, qt], m1[qt])
        P["id_sb"] = P["consts"].tile([128, 128], MMDT, tag="id_sb", name="id_sb")
        nc.sync.dma_start(P["id_sb"][:], ident)

        P["feats"] = P["dram"].tile([HWP, CO1, seq], MMDT, tag="feats", name="feats")
        P["nospill"] = (parts == "convnospill")
        if P["nospill"]:
            P["feats2"] = P["dram"].tile([n_qb, 3, 128, HWP, qb], MMDT,
                                         tag="feats2", name="feats2")

        for _rep in range(repeat):
            if parts in ("all", "conv", "convnospill", "convflat"):
                for blk in range(n_qb):
                    conv_block(nc, P, blk)
            if parts in ("all", "attn"):
                for p in range(HWP):
                    attn_problem(nc, P, p)
    nc.compile()
    return nc


def build_d2(nq=SEQ // N_CORES, repeat=1):
    """Dispatch 2: conv_out for a shard of nq frames.

    Inputs (per core):
      x2  [4, 128, nq, 100] f32r - padded attention output, ci-tile major
      w2  [128, 4, 4, 9, 128] f32r - [ci, ci_tile, co_tile, tap, co]
      b2  [128, 4] f32
    Output:
      o2  [nq, 512, 8, 8] f32
    """
    nc = bacc.Bacc("TRN2", target_bir_lowering=False, debug=False,
                   num_devices=N_CORES)
    x2 = nc.dram_tensor("x2", [4, 128, nq, 100], MMDT, kind="ExternalInput").ap()
    w2 = nc.dram_tensor("w2", [128, 4, 4, 9, 128], MMDT, kind="ExternalInput").ap()
    b2 = nc.dram_tensor("b2", [128, 4], F32, kind="ExternalInput").ap()
    o2 = nc.dram_tensor("o2", [nq, C, 8, 8], F32, kind="ExternalOutput").ap()

    with tile.TileContext(nc) as tc:
        with tc.tile_pool(name="consts", bufs=1) as consts, \
             tc.tile_pool(name="osb", bufs=1) as osb_pool, \
             tc.tile_pool(name="cps", bufs=4, space="PSUM") as conv_ps:
            w_sb = consts.tile([128, 4, 4, 9, 128], MMDT, name="w_sb")
            nc.sync.dma_start(w_sb[:], w2)
            b_sb = consts.tile([128, 4], F32, name="b_sb")
            nc.sync.dma_start(b_sb[:], b2)
            slab = consts.tile([128, 4, nq, 100], MMDT, name="slab")
            for cit in range(4):
                nc.sync.dma_start(slab[:, cit], x2[cit])
            slab5 = slab[:].rearrange("c t q (y x) -> c t q y x", y=10)

            for _rep in range(repeat):
                osbs = [osb_pool.tile([128, nq, 8, 8], F32, tag=f"osb{cot}", name=f"osb{cot}")
                        for cot in range(4)]
                for y in range(8):
                    for cot in range(4):
                        ps = conv_ps.tile([128, nq, 8], F32, tag="cps", name="cps")
                        k = 0
                        for cit in range(4):
                            for tap in range(9):
                                ddy, ddx = tap // 3 - 1, tap % 3 - 1
                                rhs = slab5[:, cit, :, y + 1 + ddy,
                                            1 + ddx:9 + ddx]
                                nc.tensor.matmul(
                                    ps[:],
                                    w_sb[:, cit, cot, tap, :],
                                    rhs,
                                    start=(k == 0), stop=(k == 35))
                                k += 1
                        nc.scalar.activation(
                            osbs[cot][:, :, y, :], ps[:],
                            mybir.ActivationFunctionType.Identity,
                            bias=b_sb[:, cot:cot + 1])
                for cot in range(4):
                    dst = o2[:, cot * 128:(cot + 1) * 128, :, :] \
                        .rearrange("q c y x -> c q (y x)")
                    nc.sync.dma_start(
                        dst, osbs[cot][:].rearrange("c q y x -> c q (y x)"))
    nc.compile()
    return nc


# ---------------- host-side data prep ----------------

def prep_d1_inputs(inp, attn_mask, agent_aware_mask, w_in, b_in):
    seq = inp.shape[1]
    x_t = np.ascontiguousarray(inp[0].transpose(1, 0, 2, 3))  # [C, seq, 8, 8]
    xp = np.zeros((C, seq, 10, 10), dtype=np.float32)
    xp[:, :, 1:9, 1:9] = x_t
    xpad = np.ascontiguousarray(xp.reshape(4, 128, seq, 100)).astype(NP_MMDT)

    ident = np.eye(128, dtype=np.float32).astype(NP_MMDT)
    n_qt = seq // 128

    maps = []
    for h in range(N_CORES):
        ch = 8 * np.arange(CO1) + h                      # conv_in channels of head h
        w = w_in[ch]                                     # [320, C, 3, 3]
        # w1[ci, cit, tap, co] = w[co, cit*128+ci, ky, kx]
        w1 = np.ascontiguousarray(
            w.reshape(CO1, 4, 128, 9).transpose(2, 1, 3, 0)).astype(np.float32)
        # fold the 1/sqrt(HD) attention scale into the q_same/q_other
        # conv channels (co 128:256) and their bias
        w1[:, :, :, 128:256] *= SCALE
        b1 = np.zeros((128, 3), dtype=np.float32)
        bh = b_in[ch].copy()
        bh[128:256] *= SCALE
        b1[:, 0] = bh[0:128]
        b1[:, 1] = bh[128:256]
        b1[0:64, 2] = bh[256:320]
        amh = np.ascontiguousarray(
            attn_mask[h].reshape(n_qt, 128, seq)).astype(np.float32)
        import ml_dtypes
        mh = agent_aware_mask[h].astype(np.float32)
        m = np.ascontiguousarray(
            mh.reshape(n_qt, 128, seq).astype(ml_dtypes.bfloat16))
        m1 = np.ascontiguousarray(
            (1.0 - mh).reshape(n_qt, 128, seq).astype(ml_dtypes.bfloat16))
        maps.append({"xpad": xpad, "w1": w1.astype(NP_MMDT), "b1": b1,
                     "am": amh.astype(NP_MMDT), "m": m, "m1": m1,
                     "ident": ident})
    return maps


def assemble_att(att_results, seq):
    """att_results: list of 8 arrays [64, 64, seq] -> padded [4,128,seq,100]."""
    A = np.zeros((64, 8, seq, 10, 10), dtype=np.float32)  # [d, head, q, 10, 10]
    for h in range(N_CORES):
        a = att_results[h].reshape(8, 8, HD, seq)         # [y, x, d, q]
        A[:, h, :, 1:9, 1:9] = a.transpose(2, 3, 0, 1)
    return np.ascontiguousarray(A.reshape(4, 128, seq, 100))


def prep_d2_weights(w_out, b_out):
    # w2[ci, cit, cot, tap, co] = w_out[cot*128+co, cit*128+ci, ky, kx]
    w2 = np.ascontiguousarray(
        w_out.reshape(4, 128, 4, 128, 9).transpose(3, 2, 0, 4, 1)).astype(np.float32)
    b2 = np.ascontiguousarray(b_out.reshape(4, 128).T).astype(np.float32)
    return w2, b2


_NC_CACHE = {}


def _get_nc(name, builder, **kw):
    key = (name, tuple(sorted(kw.items())))
    if key not in _NC_CACHE:
        _NC_CACHE[key] = builder(**kw)
    return _NC_CACHE[key]


def kernel(inp, attn_mask, agent_aware_mask, w_in, b_in, w_out, b_out):
    inp = np.asarray(inp, dtype=np.float32)
    attn_mask = np.asarray(attn_mask, dtype=np.float32)
    agent_aware_mask = np.asarray(agent_aware_mask)
    w_in = np.asarray(w_in, dtype=np.float32)
    b_in = np.asarray(b_in, dtype=np.float32)
    w_out = np.asarray(w_out, dtype=np.float32)
    b_out = np.asarray(b_out, dtype=np.float32)

    b, seq, c, h, w = inp.shape
    assert (b, c, h, w) == (1, C, H, W)

    nc1 = _get_nc("d1", build_d1, seq=seq)
    in_maps1 = prep_d1_inputs(inp, attn_mask, agent_aware_mask, w_in, b_in)
    res1 = run_bass_kernel_spmd(nc1, in_maps1, core_ids=list(range(N_CORES)))
    att_results = [res1.results[i]["att"] for i in range(N_CORES)]

    A = assemble_att(att_results, seq)
    w2, b2 = prep_d2_weights(w_out, b_out)
    nq = seq // N_CORES
    w2 = w2.astype(NP_MMDT)
    in_maps2 = [{"x2": np.ascontiguousarray(
                     A[:, :, j * nq:(j + 1) * nq, :]).astype(NP_MMDT),
                 "w2": w2, "b2": b2} for j in range(N_CORES)]
    nc2 = _get_nc("d2", build_d2, nq=nq)
    res2 = run_bass_kernel_spmd(nc2, in_maps2, core_ids=list(range(N_CORES)))
    out = np.concatenate([res2.results[j]["o2"] for j in range(N_CORES)], axis=0)
    return out.reshape(b, seq, c, h, w)

